# revision 8
# baseline (speedup 1.0000x reference)
"""GPT-2-small forward pass on 8 Trainium2 NeuronCores.

Sharding: 2 batch groups x 4-way sequence parallelism.
  - Core c in [0..3] handles batch 0, tokens [256c, 256c+256); cores 4..7 same for batch 1.
  - LN / QKV / proj / MLP are token-parallel (zero communication).
  - Per layer: one AllGather of kT and one of v within each 4-core group.
  - Pre-head: AllGather of final LN'd activations (transposed); each core then
    computes logits for all 1024 tokens of its batch x a 12576-wide vocab slice.
Matmuls run in float32r (full PE rate at N>=256, ~5e-4 rounding); LN/softmax/
residual math stays float32.

Softmax trick: the attention AV matmul's stationary operand is [v_head | ones]
(128 wide), so PSUM rows 0-63 get the unnormalized AV and rows 64-127 all get
the softmax denominator - a free partition-broadcast for the normalize step.
"""
import sys

sys.path.insert(0, "/opt/trn_rl_repo")
import numpy as np
import concourse.bass as bass
import concourse.mybir as mybir
import concourse.bacc as bacc
import concourse.tile as tile
from concourse.bass_utils import run_bass_kernel_spmd

F32 = mybir.dt.float32
F32R = mybir.dt.float32r
AF = mybir.ActivationFunctionType
ALU = mybir.AluOpType

L, C, H, HD, T, B = 6, 768, 12, 64, 1024, 2
TPC = 256            # tokens per core
NCORE = 8
GROUPS = [[0, 1, 2, 3], [4, 5, 6, 7]]
V = 50257
VP = 12576           # per-core vocab slice (4*VP = 50304 >= V, zero-padded)
CF = C // 128        # 6 feature tiles
MQK = 12             # q+k output tiles of 128 (1536 features)
MH = 24              # mlp hidden tiles (3072)
EPS = 1e-5
NKT = TPC // 128     # 2 token tiles per core
KT_ALL = T // 128    # 8 key tiles (full sequence)
VCH = 384            # head vocab chunk width


def build_nc(n_layers=L, with_head=True):
    nc = bacc.Bacc("TRN2", target_bir_lowering=False, debug=False, num_devices=NCORE)

    x0 = nc.dram_tensor("x0", [TPC, C], F32, kind="ExternalInput")
    wqk_t = nc.dram_tensor("wqk_t", [L, MQK, 128, C], F32R, kind="ExternalInput")
    wv = nc.dram_tensor("wv", [L, C, C], F32R, kind="ExternalInput")
    wproj = nc.dram_tensor("wproj", [L, C, C], F32R, kind="ExternalInput")
    w1_t = nc.dram_tensor("w1_t", [L, MH, 128, C], F32R, kind="ExternalInput")
    w2 = nc.dram_tensor("w2", [L, 4 * C, C], F32R, kind="ExternalInput")
    b1_t = nc.dram_tensor("b1_t", [L, 128, MH], F32, kind="ExternalInput")
    b2 = nc.dram_tensor("b2", [L, C], F32R, kind="ExternalInput")
    lnp = nc.dram_tensor("lnp", [4, L, 128, CF], F32, kind="ExternalInput")
    lnfp = nc.dram_tensor("lnfp", [2, 128, CF], F32, kind="ExternalInput")
    masks = nc.dram_tensor("masks", [KT_ALL, 128, TPC], F32R, kind="ExternalInput")
    ident = nc.dram_tensor("ident", [128, 128], F32, kind="ExternalInput")
    embT = nc.dram_tensor("embT", [C, VP], F32R, kind="ExternalInput")
    logits = nc.dram_tensor("logits", [T, VP], F32, kind="ExternalOutput")
    x_dbg = nc.dram_tensor("x_dbg", [TPC, C], F32, kind="ExternalOutput")

    with tile.TileContext(nc) as tc:
        with (
            tc.tile_pool(name="persist", bufs=1) as pp,
            tc.tile_pool(name="sb1", bufs=1) as sb1,
            tc.tile_pool(name="sb2", bufs=2) as sb2,
            tc.tile_pool(name="ws", bufs=3) as ws,
            tc.tile_pool(name="ps_a", bufs=2, space="PSUM") as ps_a,
            tc.tile_pool(name="ps_b", bufs=1, space="PSUM") as ps_b,
            tc.tile_pool(name="ps_av", bufs=1, space="PSUM") as ps_av,
            tc.tile_pool(name="dram", bufs=2, space="DRAM") as dram,
        ):
            # ---- persistent tiles ----
            x_sb = pp.tile([128, NKT, C], F32)          # residual stream
            nc.sync.dma_start(x_sb[:], x0.rearrange("(t p) c -> p t c", p=128))
            id_sb = pp.tile([128, 128], F32)
            nc.sync.dma_start(id_sb[:], ident[:])
            mask_sb = pp.tile([128, KT_ALL, TPC], F32R)
            nc.sync.dma_start(mask_sb[:], masks.rearrange("k p t -> p k t"))
            ones_f32 = pp.tile([128, 128], F32)
            nc.vector.memset(ones_f32[:], 1.0)
            ones1 = pp.tile([1, 128], F32R)
            nc.vector.tensor_copy(ones1[:], ones_f32[0:1, :])
            ones64 = pp.tile([128, 64], F32R)
            nc.vector.tensor_copy(ones64[:], ones_f32[:, 0:64])
            eps_sb = pp.tile([128, 1], F32)
            nc.vector.memset(eps_sb[:], EPS)

            def layernorm_T(w_ap, b_ap, out_tag):
                """LN over x_sb -> transposed f32r [128, CF, TPC] with w/b folded."""
                lnw = sb2.tile([128, CF], F32, tag="lnw")
                nc.sync.dma_start(lnw[:], w_ap)
                lnb = sb2.tile([128, CF], F32, tag="lnb")
                nc.sync.dma_start(lnb[:], b_ap)
                outT = sb2.tile([128, CF, TPC], F32R, tag=out_tag)
                for t in range(NKT):
                    stats = sb2.tile([128, 2, 6], F32, tag="ln_stats")
                    nc.vector.bn_stats(stats[:, 0, :], x_sb[:, t, 0:512])
                    nc.vector.bn_stats(stats[:, 1, :], x_sb[:, t, 512:768])
                    mv = sb2.tile([128, 2], F32, tag="ln_mv")
                    nc.vector.bn_aggr(mv[:], stats[:, :, :])
                    sd = sb2.tile([128, 1], F32, tag="ln_sd")
                    nc.scalar.activation(sd[:], mv[:, 1:2], AF.Sqrt, bias=eps_sb[:, 0:1])
                    rs = sb2.tile([128, 1], F32, tag="ln_rs")
                    nc.vector.reciprocal(rs[:], sd[:])
                    nmr = sb2.tile([128, 1], F32, tag="ln_nmr")
                    nc.vector.scalar_tensor_tensor(nmr[:], mv[:, 0:1], -1.0, rs[:],
                                                   ALU.mult, ALU.mult)
                    xn = sb2.tile([128, C], F32, tag="xn")
                    nc.scalar.activation(xn[:], x_sb[:, t, :], AF.Identity,
                                         bias=nmr[:, 0:1], scale=rs[:, 0:1])
                    for f in range(CF):
                        tp = ps_a.tile([128, 128], F32, tag="ps_mm")
                        nc.tensor.transpose(tp[:], xn[:, f * 128:(f + 1) * 128], id_sb[:])
                        nc.scalar.activation(outT[:, f, t * 128:(t + 1) * 128], tp[:],
                                             AF.Identity, bias=lnb[:, f:f + 1],
                                             scale=lnw[:, f:f + 1])
                return outT

            for l in range(n_layers):
                # ======== LN1
                xlnT = layernorm_T(lnp[0, l], lnp[1, l], "xlnT")

                # ======== QK (transposed orientation)
                qk_sb = sb1.tile([128, MQK, TPC], F32R, tag="qk_sb")
                for m in range(MQK):
                    wt = ws.tile([128, C], F32R, tag="wstream")
                    nc.sync.dma_start(wt[:], wqk_t[l, m])
                    mm = ps_a.tile([128, TPC], F32, tag="ps_mm")
                    for f in range(CF):
                        nc.tensor.matmul(mm[:], wt[:, f * 128:(f + 1) * 128],
                                         xlnT[:, f, :], start=(f == 0), stop=(f == CF - 1))
                    if m % 2 == 0:
                        nc.scalar.copy(qk_sb[:, m, :], mm[:])
                    else:
                        nc.vector.tensor_copy(qk_sb[:, m, :], mm[:])

                kt_in = dram.tile([128, CF, TPC], F32R, tag="kt_in")
                nc.sync.dma_start(kt_in[:], qk_sb[:, CF:MQK, :])
                kt_all = dram.tile([4, 128, CF, TPC], F32R, tag="kt_all")
                nc.gpsimd.collective_compute(
                    "AllGather", ALU.bypass, replica_groups=GROUPS,
                    ins=[kt_in.opt()], outs=[kt_all.opt()])

                # ======== V (natural [token, feat] orientation)
                v_own = sb1.tile([128, NKT, C], F32R, tag="v_own")
                for f in range(CF):
                    wvt = ws.tile([128, C], F32R, tag="wstream")
                    nc.sync.dma_start(wvt[:], wv[l, f * 128:(f + 1) * 128, :])
                    for t in range(NKT):
                        for ci, (c0, cw) in enumerate(((0, 512), (512, 256))):
                            vp = ps_b.tile([128, cw], F32, tag=f"ps_v{t}{ci}")
                            nc.tensor.matmul(vp[:], xlnT[:, f, t * 128:(t + 1) * 128],
                                             wvt[:, c0:c0 + cw], start=(f == 0),
                                             stop=(f == CF - 1))
                            if f == CF - 1:
                                if ci == 0:
                                    nc.scalar.copy(v_own[:, t, c0:c0 + cw], vp[:])
                                else:
                                    nc.vector.tensor_copy(v_own[:, t, c0:c0 + cw], vp[:])
                v_in = dram.tile([128, NKT, C], F32R, tag="v_in")
                nc.sync.dma_start(v_in[:], v_own[:])
                v_all = dram.tile([4, 128, NKT, C], F32R, tag="v_all")
                nc.gpsimd.collective_compute(
                    "AllGather", ALU.bypass, replica_groups=GROUPS,
                    ins=[v_in.opt()], outs=[v_all.opt()])

                # gathered K/V -> SBUF
                kT_sb = sb1.tile([128, CF, T], F32R, tag="kT_sb")
                for r in range(4):
                    nc.sync.dma_start(kT_sb[:, :, r * TPC:(r + 1) * TPC], kt_all[r])
                v_sb = sb1.tile([128, KT_ALL, C], F32R, tag="v_sb")
                for r in range(4):
                    nc.sync.dma_start(v_sb[:, 2 * r:2 * r + 2, :], v_all[r])

                # ======== attention (12 heads)
                attnT = sb1.tile([128, CF, TPC], F32R, tag="attnT")
                for h in range(H):
                    po = (h % 2) * 64
                    sl = h // 2
                    att = sb2.tile([128, KT_ALL, TPC], F32R, tag="att")
                    for kt in range(KT_ALL):
                        sp = ps_a.tile([128, TPC], F32, tag="ps_mm")
                        nc.tensor.matmul(sp[:], kT_sb[po:po + 64, sl, kt * 128:(kt + 1) * 128],
                                         qk_sb[po:po + 64, sl, :], start=True, stop=True)
                        nc.scalar.activation(att[:, kt, :], sp[:], AF.Exp, scale=0.125)
                        nc.vector.tensor_mul(att[:, kt, :], att[:, kt, :], mask_sb[:, kt, :])
                    avp = ps_av.tile([64, TPC], F32, tag="ps_av")
                    den = ps_av.tile([64, TPC], F32, tag="ps_den")
                    for kt in range(KT_ALL):
                        nc.tensor.matmul(avp[:, :], v_sb[:, kt, h * 64:(h + 1) * 64],
                                         att[:, kt, :],
                                         start=(kt == 0), stop=(kt == KT_ALL - 1))
                    for kt in range(KT_ALL):
                        nc.tensor.matmul(den[:, :], ones64[:, :], att[:, kt, :],
                                         start=(kt == 0), stop=(kt == KT_ALL - 1))
                    rec = sb2.tile([64, TPC], F32, tag="rec")
                    nc.vector.reciprocal(rec[:], den[:, :])
                    nc.vector.tensor_mul(attnT[po:po + 64, sl, :], avp[:, :], rec[:])

                # ======== proj + residual
                for f in range(CF):
                    wpt = ws.tile([128, C], F32R, tag="wstream")
                    nc.sync.dma_start(wpt[:], wproj[l, f * 128:(f + 1) * 128, :])
                    for t in range(NKT):
                        for ci, (c0, cw) in enumerate(((0, 512), (512, 256))):
                            pj = ps_b.tile([128, cw], F32, tag=f"ps_v{t}{ci}")
                            nc.tensor.matmul(pj[:], attnT[:, f, t * 128:(t + 1) * 128],
                                             wpt[:, c0:c0 + cw], start=(f == 0),
                                             stop=(f == CF - 1))
                            if f == CF - 1:
                                nc.vector.tensor_add(x_sb[:, t, c0:c0 + cw],
                                                     x_sb[:, t, c0:c0 + cw], pj[:])

                # ======== LN2 + MLP
                xln2T = layernorm_T(lnp[2, l], lnp[3, l], "xlnT")
                b1s = sb1.tile([128, MH], F32, tag="b1s")
                nc.sync.dma_start(b1s[:], b1_t[l])
                hT = sb1.tile([128, MH, TPC], F32R, tag="hT")
                for m in range(MH):
                    w1t = ws.tile([128, C], F32R, tag="wstream")
                    nc.sync.dma_start(w1t[:], w1_t[l, m])
                    hp = ps_a.tile([128, TPC], F32, tag="ps_mm")
                    for f in range(CF):
                        nc.tensor.matmul(hp[:], w1t[:, f * 128:(f + 1) * 128],
                                         xln2T[:, f, :], start=(f == 0), stop=(f == CF - 1))
                    nc.scalar.activation(hT[:, m, :], hp[:], AF.Gelu, bias=b1s[:, m:m + 1])

                b2s = sb1.tile([1, C], F32R, tag="b2s")
                nc.sync.dma_start(b2s[:], b2[l:l + 1, :])
                mlp_ps = {}
                for t in range(NKT):
                    for ci, (c0, cw) in enumerate(((0, 512), (512, 256))):
                        mlp_ps[(t, ci)] = ps_b.tile([128, cw], F32, tag=f"ps_v{t}{ci}", name=f"mlp_ps_{t}_{ci}")
                for m in range(MH):
                    w2t = ws.tile([128, C], F32R, tag="wstream")
                    nc.sync.dma_start(w2t[:], w2[l, m * 128:(m + 1) * 128, :])
                    for t in range(NKT):
                        for ci, (c0, cw) in enumerate(((0, 512), (512, 256))):
                            nc.tensor.matmul(mlp_ps[(t, ci)][:], hT[:, m, t * 128:(t + 1) * 128],
                                             w2t[:, c0:c0 + cw], start=(m == 0), stop=False)
                for t in range(NKT):
                    for ci, (c0, cw) in enumerate(((0, 512), (512, 256))):
                        nc.tensor.matmul(mlp_ps[(t, ci)][:], ones1[:, 0:128],
                                         b2s[:, c0:c0 + cw], start=False, stop=True)
                        nc.vector.tensor_add(x_sb[:, t, c0:c0 + cw],
                                             x_sb[:, t, c0:c0 + cw], mlp_ps[(t, ci)][:])

            nc.sync.dma_start(x_dbg.rearrange("(t p) c -> p t c", p=128), x_sb[:])

            if with_head:
                xfT = layernorm_T(lnfp[0], lnfp[1], "xlnT")
                xf_in = dram.tile([128, CF, TPC], F32R, tag="kt_in")
                nc.sync.dma_start(xf_in[:], xfT[:])
                xf_all = dram.tile([4, 128, CF, TPC], F32R, tag="kt_all")
                nc.gpsimd.collective_compute(
                    "AllGather", ALU.bypass, replica_groups=GROUPS,
                    ins=[xf_in.opt()], outs=[xf_all.opt()])
                xfT_all = sb1.tile([128, CF, T], F32R, tag="kT_sb")
                for r in range(4):
                    nc.sync.dma_start(xfT_all[:, :, r * TPC:(r + 1) * TPC], xf_all[r])

                embr = embT.rearrange("(f p) v -> p f v", p=128)
                vchunks = [(i * VCH, VCH) for i in range(VP // VCH)]
                if VP % VCH:
                    vchunks.append((VP - VP % VCH, VP % VCH))
                for v0, vw in vchunks:
                    et = ws.tile([128, CF, VCH], F32R, tag="wstream")
                    nc.sync.dma_start(et[:, :, 0:vw], embr[:, :, v0:v0 + vw])
                    for tt in range(KT_ALL):
                        lp = ps_b.tile([128, vw], F32, tag=f"ps_v{tt % 2}1")
                        for f in range(CF):
                            nc.tensor.matmul(lp[:], xfT_all[:, f, tt * 128:(tt + 1) * 128],
                                             et[:, f, 0:vw], start=(f == 0), stop=(f == CF - 1))
                        ot = sb2.tile([128, vw], F32, tag="head_out")
                        if tt % 2 == 0:
                            nc.scalar.copy(ot[:], lp[:])
                        else:
                            nc.vector.tensor_copy(ot[:], lp[:])
                        nc.sync.dma_start(logits[tt * 128:(tt + 1) * 128, v0:v0 + vw], ot[:])
    nc.finalize()
    return nc


_NC_CACHE = {}


def _get_nc(n_layers=L, with_head=True):
    key = (n_layers, with_head)
    if key not in _NC_CACHE:
        _NC_CACHE[key] = build_nc(n_layers, with_head)
    return _NC_CACHE[key]


def prepare_inputs(idx, emb, pos_emb, Wqkv, Wproj, ln1_w, ln1_b, ln2_w, ln2_b,
                   W1, b1, W2, b2, lnf_w, lnf_b):
    """Host-side sharding: returns in_maps list of 8 dicts."""
    f32 = np.float32
    emb = np.asarray(emb, f32); pos_emb = np.asarray(pos_emb, f32)
    Wqkv = np.asarray(Wqkv, f32); Wproj = np.ascontiguousarray(np.asarray(Wproj, f32))
    W1 = np.asarray(W1, f32); W2 = np.ascontiguousarray(np.asarray(W2, f32))
    b1 = np.asarray(b1, f32); b2 = np.ascontiguousarray(np.asarray(b2, f32))

    wqk_t = np.ascontiguousarray(
        Wqkv[:, :, :1536].reshape(L, CF, 128, MQK, 128).transpose(0, 3, 2, 1, 4)
        .reshape(L, MQK, 128, C))
    wv = np.ascontiguousarray(Wqkv[:, :, 1536:])
    w1_t = np.ascontiguousarray(
        W1.reshape(L, CF, 128, MH, 128).transpose(0, 3, 2, 1, 4).reshape(L, MH, 128, C))
    b1_t = np.ascontiguousarray(b1.reshape(L, MH, 128).transpose(0, 2, 1))
    lnp = np.ascontiguousarray(np.stack([
        np.asarray(p, f32).reshape(L, CF, 128).transpose(0, 2, 1)
        for p in (ln1_w, ln1_b, ln2_w, ln2_b)]))
    lnfp = np.ascontiguousarray(
        np.stack([np.asarray(p, f32).reshape(CF, 128).T for p in (lnf_w, lnf_b)]))
    identity = np.eye(128, dtype=f32)

    embT_pad = np.zeros((C, 4 * VP), f32)
    embT_pad[:, :V] = emb.T

    x0_full = np.asarray(emb[np.asarray(idx)] + pos_emb[None, :T, :], f32)

    in_maps = []
    for core in range(NCORE):
        g, r = divmod(core, 4)
        qs = r * TPC
        kidx = np.arange(T)[:, None]
        mask = (kidx <= (qs + np.arange(TPC))[None, :]).astype(f32)  # [T, TPC]
        masks_c = np.ascontiguousarray(mask.reshape(KT_ALL, 128, TPC))
        in_maps.append({
            "x0": np.ascontiguousarray(x0_full[g, qs:qs + TPC]),
            "wqk_t": wqk_t, "wv": wv, "wproj": Wproj,
            "w1_t": w1_t, "w2": W2,
            "b1_t": b1_t, "b2": b2, "lnp": lnp, "lnfp": lnfp,
            "masks": masks_c, "ident": identity,
            "embT": np.ascontiguousarray(embT_pad[:, r * VP:(r + 1) * VP]),
        })
    return in_maps


def run(inputs, n_layers=L, with_head=True, trace=False):
    nc = _get_nc(n_layers, with_head)
    in_maps = prepare_inputs(**inputs)
    return run_bass_kernel_spmd(nc, in_maps, list(range(NCORE)), trace=trace)


def kernel(**inputs) -> np.ndarray:
    res = run(inputs)
    out = np.empty((B, T, V), np.float32)
    for g in range(B):
        full = np.concatenate([res.results[g * 4 + r]["logits"] for r in range(4)], axis=1)
        out[g] = full[:, :V]
    return out
